# revision 22
# baseline (speedup 1.0000x reference)
"""Multi-head attention (B=2, S=4096, D=512, H=8, DR=64) on 8 trn2 NeuronCores.

Sharding: core c -> batch b = c // 4, head-pair hp = c % 4 (heads 2*hp, 2*hp+1).
Each core computes, for its batch and its two heads:
    q/k/v projections, flash-style attention (scores kept on-chip in
    transposed [t, s] orientation so softmax row-sums come from a fused
    ones-column in the AV matmul), and the partial output projection
    y_part = concat(out_h0, out_h1) @ Wo[rows of those heads].
Host sums the 4 partials per batch and adds the bias.

Matmul operands are cast to bf16 on-chip (fp32 matmuls on trn2 run as two
LOW/HIGH passes with an un-hidden LDWEIGHTS between them - ~3x the cost of a
bf16 matmul). All accumulation stays fp32 in PSUM; exp() runs on the fp32
scores; softmax denominators are exact sums of the quantized bf16 weights, so
the attention rows still sum to 1.

The input pipeline (x load/cast/transpose + q/k/v projections + v transpose)
is emitted in 8 groups of 512 s-columns, interleaved with the first
attention block's t-loop, so the ScalarE exp stream starts after one group
instead of after the whole prologue. Per-group SBUF tiles give the Tile
scheduler the dataflow to overlap group g+1's production with attention over
group g.
"""

import sys

for _p in ("/opt/trn_rl_repo", "/root/.axon_site/_ro/trn_rl_repo"):
    if _p not in sys.path:
        sys.path.insert(0, _p)

import numpy as np
from contextlib import ExitStack

import concourse.bass as bass
import concourse.tile as tile
import concourse.mybir as mybir
from concourse.bass_utils import run_bass_kernel_spmd
from concourse.masks import make_identity

B, S, D = 2, 4096, 512
H, DR = 8, 64
P = 128
NT = S // P          # 32 t-tiles (also s-tiles)
SBW = 512            # s-block width
NSB = S // SBW       # 8 s-blocks / t-groups
DC = D // P          # 4 d-chunks
GT = SBW // P        # 4 t-tiles per group
N_CORES = 8
FP32 = mybir.dt.float32
BF16 = mybir.dt.bfloat16

_drain_patched = False


def _patch_tile_drain():
    """This walrus build rejects >1 sync wait on one instruction, which breaks
    TileContext's kernel-tail drain. Spread the waits over nop instructions
    emitted just before the drain."""
    global _drain_patched
    if _drain_patched:
        return
    _drain_patched = True

    def patched(self, tick_clock, wait_clock):
        nop0 = self.nc.sync.nop()
        wait_clock.add_sem_waits(
            nop0.ins, tile.ScopedClock({None: tick_clock.global_clock})
        )
        si = nop0.ins.sync_info
        waits = list(si.on_wait) if si is not None else []
        if waits:
            nop0.ins.sync_info = mybir.SyncInfo(on_wait=waits[:1], on_update=[])
            for w in waits[1:]:
                nop = self.nc.sync.nop()
                nop.ins.sync_info = mybir.SyncInfo(on_wait=[w], on_update=[])
        self.nc.sync.drain()
        self.nc.all_engine_barrier()
        popped = self.nc._tile_sem_poison_stack.pop()
        assert popped is self._sem_poison
        self.nc.clear_and_free_semaphores(list(self.sems.allocated().values()))
        self.nc.all_engine_barrier()

    tile.TileContext._drain_and_barrier = patched


# This walrus build supports only one sync-wait slot per instruction, while
# Tile's sem-assigner attaches up to ~3. Spread the excess onto NoOp
# instructions inserted immediately before the owning instruction (same
# engine, so the stall point is identical and no deadlock can be introduced).
_WAIT_LIMIT = 1
_SKIP_OPCODES = {"AllEngineBarrier", "EventSemaphore", "Call"}


def _split_sync_waits(nc: bass.Bass):
    noop_cls = getattr(mybir, "InstNoOp", None)
    if noop_cls is None:
        import bass_rust

        noop_cls = bass_rust.InstNoOp
    counter = [0]
    for f in nc.m.functions:
        for blk in f.blocks:
            insts = blk.instructions
            new_list = []
            changed = False
            for inst in insts:
                si = inst.sync_info
                waits = list(si.on_wait) if si is not None and si.on_wait else []
                if (
                    len(waits) > _WAIT_LIMIT
                    and inst.opcode not in _SKIP_OPCODES
                    and all(w.sync_type == "semaphore" for w in waits)
                ):
                    excess = waits[: len(waits) - _WAIT_LIMIT]
                    keep = waits[len(waits) - _WAIT_LIMIT :]
                    for w in excess:
                        counter[0] += 1
                        new_list.append(
                            noop_cls(
                                name=f"I-waitsplit-{counter[0]}",
                                engine=inst.engine,
                                debug=inst.debug,
                                ins=[],
                                outs=[],
                                sync_info=mybir.SyncInfo(
                                    on_wait=[w], on_update=[]
                                ),
                            )
                        )
                    inst.sync_info = mybir.SyncInfo(
                        on_wait=keep, on_update=list(si.on_update or [])
                    )
                    changed = True
                new_list.append(inst)
            if changed:
                insts.clear()
                insts.extend(new_list)


def _build_program() -> bass.Bass:
    _patch_tile_drain()
    nc = bass.Bass()

    xt_d = nc.declare_dram_parameter("xt", [D, S], BF16, isOutput=False)
    wq_d = nc.declare_dram_parameter("wq", [D, P], BF16, isOutput=False)
    wk_d = nc.declare_dram_parameter("wk", [D, P], BF16, isOutput=False)
    wv_d = nc.declare_dram_parameter("wv", [D, P], BF16, isOutput=False)
    wo_d = nc.declare_dram_parameter("wo", [P, D], BF16, isOutput=False)
    y_d = nc.declare_dram_parameter("y", [S, D], FP32, isOutput=True)

    with tile.TileContext(nc) as tc, ExitStack() as ctx:
        consts = ctx.enter_context(tc.tile_pool(name="consts", bufs=1))
        wpool = ctx.enter_context(tc.tile_pool(name="weights", bufs=1))
        big = ctx.enter_context(tc.tile_pool(name="big", bufs=1))
        aux = ctx.enter_context(tc.tile_pool(name="aux", bufs=2, space="PSUM"))
        psp = ctx.enter_context(tc.tile_pool(name="ps", bufs=2, space="PSUM"))
        pop = ctx.enter_context(tc.tile_pool(name="po", bufs=2, space="PSUM"))
        epool = ctx.enter_context(tc.tile_pool(name="exp", bufs=11))
        spool = ctx.enter_context(tc.tile_pool(name="small", bufs=4))
        opool = ctx.enter_context(tc.tile_pool(name="osb", bufs=3))
        ypool = ctx.enter_context(tc.tile_pool(name="yout", bufs=3))

        ones64 = consts.tile([1, 64], FP32)
        nc.vector.memset(ones64[:], 1.0)

        # PE warm-up: dense junk matmuls during the initial DMA-bound window
        # keep the HAM clock-gate at 8/8 so the first real matmuls run at
        # 2.4 GHz instead of 1.2 GHz.
        warm = consts.tile([P, D], BF16)
        nc.vector.memset(warm[:], 0.0)
        pw = aux.tile([P, D], FP32, tag="aux", name="pw")
        for _ in range(24):
            nc.tensor.matmul(
                pw[:], warm[:, 0:P], warm[:], start=True, stop=True
            )

        # Weights in bf16; w*_b[p, c*128 + e] = W[c*128 + p, e]
        wq_b = wpool.tile([P, D], BF16)
        wk_b = wpool.tile([P, D], BF16)
        wv_b = wpool.tile([P, D], BF16)
        wo_b = wpool.tile([P, D], BF16)
        for w_b, w_dram in ((wq_b, wq_d), (wk_b, wk_d), (wv_b, wv_d)):
            nc.gpsimd.dma_start(
                w_b[:].rearrange("p (c e) -> p c e", c=DC),
                w_dram[:].rearrange("(c p) e -> p c e", p=P),
            )
        nc.gpsimd.dma_start(wo_b[:], wo_d[:])

        # Per-group persistent tiles (bufs=NSB so every group stays live).
        # xT_g[g][p, c*512 + j] = x[g*512 + j, c*128 + p]
        xtp = ctx.enter_context(tc.tile_pool(name="xtg", bufs=NSB))
        ktp = ctx.enter_context(tc.tile_pool(name="ktg", bufs=NSB))
        qtp = ctx.enter_context(tc.tile_pool(name="qtg", bufs=NSB))
        vtp = ctx.enter_context(tc.tile_pool(name="vtg", bufs=2))
        vsp = ctx.enter_context(tc.tile_pool(name="vsg", bufs=NSB))
        xT_g = [None] * NSB
        kT_g = [None] * NSB   # [e(h0|h1), 512 t-cols]
        qT_g = [None] * NSB   # [e(h0|h1), 512 s-cols]
        v_g = [None] * NSB    # per t-tile in group: [t, 65*2] = [vh0|1 | vh1|1]

        def produce_qk(g):
            xt = xtp.tile([P, DC * SBW], BF16, tag="xt")
            xT_g[g] = xt
            for c in range(DC):
                eng = nc.sync if c % 2 == 0 else nc.scalar
                eng.dma_start(
                    xt[:, c * SBW : (c + 1) * SBW],
                    xt_d[c * P : (c + 1) * P, g * SBW : (g + 1) * SBW],
                )
            kt = ktp.tile([P, SBW], BF16, tag="kt")
            qt = qtp.tile([P, SBW], BF16, tag="qt")
            kT_g[g] = kt
            qT_g[g] = qt
            for w_b, dstT in ((wq_b, qt), (wk_b, kt)):
                pp = aux.tile([P, SBW], FP32, tag="aux")
                for c in range(DC):
                    nc.tensor.matmul(
                        pp[:],
                        w_b[:, c * P : (c + 1) * P],
                        xt[:, c * SBW : (c + 1) * SBW],
                        start=(c == 0),
                        stop=(c == DC - 1),
                    )
                nc.vector.tensor_copy(dstT[:], pp[:])

        def produce_v(g):
            xt = xT_g[g]
            vs = vsp.tile([P, GT * 130], BF16, tag="vs")
            v_g[g] = vs
            for j in range(GT):
                pv = aux.tile([P, P], FP32, tag="aux")
                for c in range(DC):
                    nc.tensor.matmul(
                        pv[:],
                        xt[:, c * SBW + j * P : c * SBW + (j + 1) * P],
                        wv_b[:, c * P : (c + 1) * P],
                        start=(c == 0),
                        stop=(c == DC - 1),
                    )
                dstv = vs[:, j * 130 : j * 130 + 130].rearrange(
                    "p (h q) -> p h q", h=2
                )[:, :, 0:64]
                nc.vector.tensor_copy(
                    dstv, pv[:].rearrange("p (h q) -> p h q", h=2)
                )
            ones_cols = vs[:].rearrange("p (t q) -> p t q", t=GT)[:, :, 64:130:65]
            nc.vector.memset(ones_cols, 1.0)

        def produce_group(g):
            produce_qk(g)
            produce_v(g)

        # ---- attention + output projection ----
        # Epilogue part 1 (right after a block's t-loop): copy softmax sums
        # and unnormalized bf16 outputs out of PSUM so the po accumulators
        # free immediately. Part 2 (deferred into the next block's t-loop):
        # broadcast sums via PE, one exact reciprocal, normalize, project.
        DEFER_ITERS = 8
        pending = [None]

        def epilogue_part1(sb, po0, po1):
            s0 = spool.tile([1, SBW], FP32, tag="r")
            s1 = spool.tile([1, SBW], FP32, tag="r")
            nc.vector.tensor_copy(s0[:], po0[64:65, :])
            nc.vector.tensor_copy(s1[:], po1[64:65, :])
            osb_u = opool.tile([P, SBW], BF16, tag="osb")
            nc.vector.tensor_copy(osb_u[0:64, :], po0[0:64, :])
            nc.vector.tensor_copy(osb_u[64:128, :], po1[0:64, :])
            pending[0] = (sb, s0, s1, osb_u)

        def epilogue_part2():
            if pending[0] is None:
                return
            sb, s0, s1, osb_u = pending[0]
            pending[0] = None
            pb_t = aux.tile([P, SBW], FP32, tag="aux")
            nc.tensor.matmul(
                pb_t[0:64, :], ones64[:], s0[:],
                start=True, stop=True, tile_position=(0, 0),
            )
            nc.tensor.matmul(
                pb_t[64:128, :], ones64[:], s1[:],
                start=True, stop=True, tile_position=(0, 64),
            )
            bc = spool.tile([P, SBW], FP32, tag="bc")
            nc.vector.tensor_copy(bc[:], pb_t[:])
            osb = opool.tile([P, SBW], BF16, tag="osb")
            for st in range(SBW // P):
                sl = slice(st * P, (st + 1) * P)
                rc = spool.tile([P, P], FP32, tag="rc")
                nc.vector.reciprocal(rc[:], bc[:, sl])
                rcb = spool.tile([P, P], BF16, tag="rcb")
                nc.vector.tensor_copy(rcb[:], rc[:])
                nc.vector.tensor_mul(osb[:, sl], osb_u[:, sl], rcb[:])
                py_t = aux.tile([P, D], FP32, tag="aux")
                nc.tensor.matmul(
                    py_t[:],
                    osb[:, sl],
                    wo_b[:],
                    start=True,
                    stop=True,
                )
                ysb = ypool.tile([P, D], FP32, tag="y")
                nc.vector.tensor_copy(ysb[:], py_t[:])
                row = (sb * (SBW // P) + st) * P
                eng = nc.sync if st % 2 == 0 else nc.scalar
                eng.dma_start(y_d[row : row + P, :], ysb[:])

        produce_group(0)
        produce_group(1)

        PREF = 3
        SPLICE_QK = {2: 2, 6: 3, 11: 4, 15: 5, 20: 6, 24: 7}
        SPLICE_V = {4: 2, 8: 3, 13: 4, 17: 5, 22: 6, 26: 7}
        NQ = NSB * NT
        po_cur = [None, None]
        ex_q = {}
        for q in range(NQ + PREF):
            if q < NQ:
                sb, tt = q // NT, q % NT
                g, j = tt // GT, tt % GT
                if sb == 0:
                    if tt in SPLICE_QK:
                        produce_qk(SPLICE_QK[tt])
                    if tt in SPLICE_V:
                        produce_v(SPLICE_V[tt])
                kt, qt = kT_g[g], qT_g[sb]
                ps_t = psp.tile([P, 2 * SBW], FP32, tag="ps")
                nc.tensor.matmul(
                    ps_t[:, 0:SBW],
                    kt[0:64, j * P : (j + 1) * P],
                    qt[0:64, :],
                    start=True,
                    stop=True,
                    tile_position=(0, 0),
                )
                nc.tensor.matmul(
                    ps_t[:, SBW : 2 * SBW],
                    kt[64:128, j * P : (j + 1) * P],
                    qt[64:128, :],
                    start=True,
                    stop=True,
                    tile_position=(64, 0),
                )
                ex = epool.tile([P, 2 * SBW], BF16, tag="exp")
                nc.scalar.activation(
                    ex[:], ps_t[:], mybir.ActivationFunctionType.Exp,
                    scale=float(1.0 / np.sqrt(DR)),
                )
                ex_q[q] = ex
                if tt == DEFER_ITERS:
                    epilogue_part2()
            if q >= PREF:
                qa = q - PREF
                sba, ta = qa // NT, qa % NT
                ga, ja = ta // GT, ta % GT
                if ta == 0:
                    po_cur[0] = pop.tile([65, SBW], FP32, tag="po", name="po0")
                    po_cur[1] = pop.tile([65, SBW], FP32, tag="po", name="po1")
                po0, po1 = po_cur
                vs, ex = v_g[ga], ex_q.pop(qa)
                nc.tensor.matmul(
                    po0[:],
                    vs[:, ja * 130 : ja * 130 + 65],
                    ex[:, 0:SBW],
                    start=(ta == 0),
                    stop=(ta == NT - 1),
                )
                nc.tensor.matmul(
                    po1[:],
                    vs[:, ja * 130 + 65 : ja * 130 + 130],
                    ex[:, SBW : 2 * SBW],
                    start=(ta == 0),
                    stop=(ta == NT - 1),
                )
                if ta == NT - 1:
                    epilogue_part1(sba, po0, po1)
        epilogue_part2()

    _split_sync_waits(nc)
    return nc


_program = None


def _get_program():
    global _program
    if _program is None:
        _program = _build_program()
    return _program


def _make_in_maps(x, Wq, Wk, Wv, Wo):
    import ml_dtypes

    bf16 = ml_dtypes.bfloat16
    xts = [np.ascontiguousarray(x[b].T).astype(bf16) for b in range(B)]
    in_maps = []
    for c in range(N_CORES):
        b = c // 4
        hp = c % 4
        h0, h1 = 2 * hp, 2 * hp + 1
        in_maps.append(
            {
                "xt": xts[b],
                "wq": np.ascontiguousarray(
                    np.concatenate([Wq[h0], Wq[h1]], axis=1)
                ).astype(bf16),
                "wk": np.ascontiguousarray(
                    np.concatenate([Wk[h0], Wk[h1]], axis=1)
                ).astype(bf16),
                "wv": np.ascontiguousarray(
                    np.concatenate([Wv[h0], Wv[h1]], axis=1)
                ).astype(bf16),
                "wo": np.ascontiguousarray(Wo[hp * 128 : (hp + 1) * 128]).astype(
                    bf16
                ),
            }
        )
    return in_maps


def kernel(**inputs) -> np.ndarray:
    x = np.asarray(inputs["x"], dtype=np.float32)
    Wq = np.asarray(inputs["Wq"], dtype=np.float32)
    Wk = np.asarray(inputs["Wk"], dtype=np.float32)
    Wv = np.asarray(inputs["Wv"], dtype=np.float32)
    Wo = np.asarray(inputs["Wo"], dtype=np.float32)
    bo = np.asarray(inputs["bo"], dtype=np.float32)

    nc = _get_program()
    in_maps = _make_in_maps(x, Wq, Wk, Wv, Wo)
    res = run_bass_kernel_spmd(nc, in_maps, list(range(N_CORES)))

    y = np.zeros((B, S, D), dtype=np.float32)
    for c in range(N_CORES):
        y[c // 4] += res.results[c]["y"]
    y += bo[None, None, :]
    return y


# revision 23
# speedup vs baseline: 1.0276x; 1.0276x over previous
"""Multi-head attention (B=2, S=4096, D=512, H=8, DR=64) on 8 trn2 NeuronCores.

Sharding: core c -> batch b = c // 4, head-pair hp = c % 4 (heads 2*hp, 2*hp+1).
Each core computes, for its batch and its two heads:
    q/k/v projections, flash-style attention (scores kept on-chip in
    transposed [t, s] orientation so softmax row-sums come from a fused
    ones-column in the AV matmul), and the partial output projection
    y_part = concat(out_h0, out_h1) @ Wo[rows of those heads].
Host sums the 4 partials per batch and adds the bias.

Matmul operands are cast to bf16 on-chip (fp32 matmuls on trn2 run as two
LOW/HIGH passes with an un-hidden LDWEIGHTS between them - ~3x the cost of a
bf16 matmul). All accumulation stays fp32 in PSUM; exp() runs on the fp32
scores; softmax denominators are exact sums of the quantized bf16 weights, so
the attention rows still sum to 1.

The input pipeline (x load/cast/transpose + q/k/v projections + v transpose)
is emitted in 8 groups of 512 s-columns, interleaved with the first
attention block's t-loop, so the ScalarE exp stream starts after one group
instead of after the whole prologue. Per-group SBUF tiles give the Tile
scheduler the dataflow to overlap group g+1's production with attention over
group g.
"""

import sys

for _p in ("/opt/trn_rl_repo", "/root/.axon_site/_ro/trn_rl_repo"):
    if _p not in sys.path:
        sys.path.insert(0, _p)

import numpy as np
from contextlib import ExitStack

import concourse.bass as bass
import concourse.tile as tile
import concourse.mybir as mybir
from concourse.bass_utils import run_bass_kernel_spmd
from concourse.masks import make_identity

B, S, D = 2, 4096, 512
H, DR = 8, 64
P = 128
NT = S // P          # 32 t-tiles (also s-tiles)
SBW = 512            # s-block width
NSB = S // SBW       # 8 s-blocks / t-groups
DC = D // P          # 4 d-chunks
GT = SBW // P        # 4 t-tiles per group
N_CORES = 8
FP32 = mybir.dt.float32
BF16 = mybir.dt.bfloat16

_drain_patched = False


def _patch_tile_drain():
    """This walrus build rejects >1 sync wait on one instruction, which breaks
    TileContext's kernel-tail drain. Spread the waits over nop instructions
    emitted just before the drain."""
    global _drain_patched
    if _drain_patched:
        return
    _drain_patched = True

    def patched(self, tick_clock, wait_clock):
        nop0 = self.nc.sync.nop()
        wait_clock.add_sem_waits(
            nop0.ins, tile.ScopedClock({None: tick_clock.global_clock})
        )
        si = nop0.ins.sync_info
        waits = list(si.on_wait) if si is not None else []
        if waits:
            nop0.ins.sync_info = mybir.SyncInfo(on_wait=waits[:1], on_update=[])
            for w in waits[1:]:
                nop = self.nc.sync.nop()
                nop.ins.sync_info = mybir.SyncInfo(on_wait=[w], on_update=[])
        self.nc.sync.drain()
        self.nc.all_engine_barrier()
        popped = self.nc._tile_sem_poison_stack.pop()
        assert popped is self._sem_poison
        self.nc.clear_and_free_semaphores(list(self.sems.allocated().values()))
        self.nc.all_engine_barrier()

    tile.TileContext._drain_and_barrier = patched


# This walrus build supports only one sync-wait slot per instruction, while
# Tile's sem-assigner attaches up to ~3. Spread the excess onto NoOp
# instructions inserted immediately before the owning instruction (same
# engine, so the stall point is identical and no deadlock can be introduced).
_WAIT_LIMIT = 1
_SKIP_OPCODES = {"AllEngineBarrier", "EventSemaphore", "Call"}


def _split_sync_waits(nc: bass.Bass):
    noop_cls = getattr(mybir, "InstNoOp", None)
    if noop_cls is None:
        import bass_rust

        noop_cls = bass_rust.InstNoOp
    counter = [0]
    for f in nc.m.functions:
        for blk in f.blocks:
            insts = blk.instructions
            new_list = []
            changed = False
            for inst in insts:
                si = inst.sync_info
                waits = list(si.on_wait) if si is not None and si.on_wait else []
                if (
                    len(waits) > _WAIT_LIMIT
                    and inst.opcode not in _SKIP_OPCODES
                    and all(w.sync_type == "semaphore" for w in waits)
                ):
                    excess = waits[: len(waits) - _WAIT_LIMIT]
                    keep = waits[len(waits) - _WAIT_LIMIT :]
                    for w in excess:
                        counter[0] += 1
                        new_list.append(
                            noop_cls(
                                name=f"I-waitsplit-{counter[0]}",
                                engine=inst.engine,
                                debug=inst.debug,
                                ins=[],
                                outs=[],
                                sync_info=mybir.SyncInfo(
                                    on_wait=[w], on_update=[]
                                ),
                            )
                        )
                    inst.sync_info = mybir.SyncInfo(
                        on_wait=keep, on_update=list(si.on_update or [])
                    )
                    changed = True
                new_list.append(inst)
            if changed:
                insts.clear()
                insts.extend(new_list)


def _build_program() -> bass.Bass:
    _patch_tile_drain()
    nc = bass.Bass()

    xt_d = nc.declare_dram_parameter("xt", [D, S], BF16, isOutput=False)
    wq_d = nc.declare_dram_parameter("wq", [D, P], BF16, isOutput=False)
    wk_d = nc.declare_dram_parameter("wk", [D, P], BF16, isOutput=False)
    wv_d = nc.declare_dram_parameter("wv", [D, P], BF16, isOutput=False)
    wo_d = nc.declare_dram_parameter("wo", [P, D], BF16, isOutput=False)
    y_d = nc.declare_dram_parameter("y", [S, D], FP32, isOutput=True)

    with tile.TileContext(nc) as tc, ExitStack() as ctx:
        consts = ctx.enter_context(tc.tile_pool(name="consts", bufs=1))
        wpool = ctx.enter_context(tc.tile_pool(name="weights", bufs=1))
        big = ctx.enter_context(tc.tile_pool(name="big", bufs=1))
        aux = ctx.enter_context(tc.tile_pool(name="aux", bufs=2, space="PSUM"))
        psp = ctx.enter_context(tc.tile_pool(name="ps", bufs=2, space="PSUM"))
        pop = ctx.enter_context(tc.tile_pool(name="po", bufs=2, space="PSUM"))
        epool = ctx.enter_context(tc.tile_pool(name="exp", bufs=11))
        spool = ctx.enter_context(tc.tile_pool(name="small", bufs=4))
        opool = ctx.enter_context(tc.tile_pool(name="osb", bufs=3))
        ypool = ctx.enter_context(tc.tile_pool(name="yout", bufs=3))

        ones64 = consts.tile([1, 64], FP32)
        nc.vector.memset(ones64[:], 1.0)

        # PE warm-up: dense junk matmuls during the initial DMA-bound window
        # keep the HAM clock-gate at 8/8 so the first real matmuls run at
        # 2.4 GHz instead of 1.2 GHz.
        warm = consts.tile([P, D], BF16)
        nc.vector.memset(warm[:], 0.0)
        pw = aux.tile([P, D], FP32, tag="aux", name="pw")
        for _ in range(24):
            nc.tensor.matmul(
                pw[:], warm[:, 0:P], warm[:], start=True, stop=True
            )

        # Weights in bf16; w*_b[p, c*128 + e] = W[c*128 + p, e]
        wq_b = wpool.tile([P, D], BF16)
        wk_b = wpool.tile([P, D], BF16)
        wv_b = wpool.tile([P, D], BF16)
        wo_b = wpool.tile([P, D], BF16)
        for w_b, w_dram in ((wq_b, wq_d), (wk_b, wk_d), (wv_b, wv_d)):
            nc.gpsimd.dma_start(
                w_b[:].rearrange("p (c e) -> p c e", c=DC),
                w_dram[:].rearrange("(c p) e -> p c e", p=P),
            )
        nc.gpsimd.dma_start(wo_b[:], wo_d[:])

        # Per-group persistent tiles (bufs=NSB so every group stays live).
        # xT_g[g][p, c*512 + j] = x[g*512 + j, c*128 + p]
        xtp = ctx.enter_context(tc.tile_pool(name="xtg", bufs=NSB))
        ktp = ctx.enter_context(tc.tile_pool(name="ktg", bufs=NSB))
        qtp = ctx.enter_context(tc.tile_pool(name="qtg", bufs=NSB))
        vtp = ctx.enter_context(tc.tile_pool(name="vtg", bufs=2))
        vsp = ctx.enter_context(tc.tile_pool(name="vsg", bufs=NSB))
        xT_g = [None] * NSB
        kT_g = [None] * NSB   # [e(h0|h1), 512 t-cols]
        qT_g = [None] * NSB   # [e(h0|h1), 512 s-cols]
        v_g = [None] * NSB    # per t-tile in group: [t, 65*2] = [vh0|1 | vh1|1]

        def produce_qk(g):
            xt = xtp.tile([P, DC * SBW], BF16, tag="xt")
            xT_g[g] = xt
            for c in range(DC):
                eng = nc.sync
                eng.dma_start(
                    xt[:, c * SBW : (c + 1) * SBW],
                    xt_d[c * P : (c + 1) * P, g * SBW : (g + 1) * SBW],
                )
            kt = ktp.tile([P, SBW], BF16, tag="kt")
            qt = qtp.tile([P, SBW], BF16, tag="qt")
            kT_g[g] = kt
            qT_g[g] = qt
            for w_b, dstT in ((wq_b, qt), (wk_b, kt)):
                pp = aux.tile([P, SBW], FP32, tag="aux")
                for c in range(DC):
                    nc.tensor.matmul(
                        pp[:],
                        w_b[:, c * P : (c + 1) * P],
                        xt[:, c * SBW : (c + 1) * SBW],
                        start=(c == 0),
                        stop=(c == DC - 1),
                    )
                nc.vector.tensor_copy(dstT[:], pp[:])

        def produce_v(g):
            xt = xT_g[g]
            vs = vsp.tile([P, GT * 130], BF16, tag="vs")
            v_g[g] = vs
            for j in range(GT):
                pv = aux.tile([P, P], FP32, tag="aux")
                for c in range(DC):
                    nc.tensor.matmul(
                        pv[:],
                        xt[:, c * SBW + j * P : c * SBW + (j + 1) * P],
                        wv_b[:, c * P : (c + 1) * P],
                        start=(c == 0),
                        stop=(c == DC - 1),
                    )
                dstv = vs[:, j * 130 : j * 130 + 130].rearrange(
                    "p (h q) -> p h q", h=2
                )[:, :, 0:64]
                nc.vector.tensor_copy(
                    dstv, pv[:].rearrange("p (h q) -> p h q", h=2)
                )
            ones_cols = vs[:].rearrange("p (t q) -> p t q", t=GT)[:, :, 64:130:65]
            nc.vector.memset(ones_cols, 1.0)

        def produce_group(g):
            produce_qk(g)
            produce_v(g)

        # ---- attention + output projection ----
        # Epilogue part 1 (right after a block's t-loop): copy softmax sums
        # and unnormalized bf16 outputs out of PSUM so the po accumulators
        # free immediately. Part 2 (deferred into the next block's t-loop):
        # broadcast sums via PE, one exact reciprocal, normalize, project.
        DEFER_ITERS = 8
        pending = [None]

        def epilogue_part1(sb, po0, po1):
            s0 = spool.tile([1, SBW], FP32, tag="r")
            s1 = spool.tile([1, SBW], FP32, tag="r")
            nc.vector.tensor_copy(s0[:], po0[64:65, :])
            nc.vector.tensor_copy(s1[:], po1[64:65, :])
            osb_u = opool.tile([P, SBW], BF16, tag="osb")
            nc.vector.tensor_copy(osb_u[0:64, :], po0[0:64, :])
            nc.vector.tensor_copy(osb_u[64:128, :], po1[0:64, :])
            pending[0] = (sb, s0, s1, osb_u)

        def epilogue_part2():
            if pending[0] is None:
                return
            sb, s0, s1, osb_u = pending[0]
            pending[0] = None
            pb_t = aux.tile([P, SBW], FP32, tag="aux")
            nc.tensor.matmul(
                pb_t[0:64, :], ones64[:], s0[:],
                start=True, stop=True, tile_position=(0, 0),
            )
            nc.tensor.matmul(
                pb_t[64:128, :], ones64[:], s1[:],
                start=True, stop=True, tile_position=(0, 64),
            )
            bc = spool.tile([P, SBW], FP32, tag="bc")
            nc.vector.tensor_copy(bc[:], pb_t[:])
            osb = opool.tile([P, SBW], BF16, tag="osb")
            for st in range(SBW // P):
                sl = slice(st * P, (st + 1) * P)
                rc = spool.tile([P, P], FP32, tag="rc")
                nc.vector.reciprocal(rc[:], bc[:, sl])
                rcb = spool.tile([P, P], BF16, tag="rcb")
                nc.vector.tensor_copy(rcb[:], rc[:])
                nc.vector.tensor_mul(osb[:, sl], osb_u[:, sl], rcb[:])
                py_t = aux.tile([P, D], FP32, tag="aux")
                nc.tensor.matmul(
                    py_t[:],
                    osb[:, sl],
                    wo_b[:],
                    start=True,
                    stop=True,
                )
                ysb = ypool.tile([P, D], FP32, tag="y")
                nc.vector.tensor_copy(ysb[:], py_t[:])
                row = (sb * (SBW // P) + st) * P
                nc.sync.dma_start(y_d[row : row + P, :], ysb[:])

        produce_group(0)
        produce_group(1)

        PREF = 3
        SPLICE_QK = {2: 2, 6: 3, 11: 4, 15: 5, 20: 6, 24: 7}
        SPLICE_V = {4: 2, 8: 3, 13: 4, 17: 5, 22: 6, 26: 7}
        NQ = NSB * NT
        po_cur = [None, None]
        ex_q = {}
        for q in range(NQ + PREF):
            if q < NQ:
                sb, tt = q // NT, q % NT
                g, j = tt // GT, tt % GT
                if sb == 0:
                    if tt in SPLICE_QK:
                        produce_qk(SPLICE_QK[tt])
                    if tt in SPLICE_V:
                        produce_v(SPLICE_V[tt])
                kt, qt = kT_g[g], qT_g[sb]
                ps_t = psp.tile([P, 2 * SBW], FP32, tag="ps")
                nc.tensor.matmul(
                    ps_t[:, 0:SBW],
                    kt[0:64, j * P : (j + 1) * P],
                    qt[0:64, :],
                    start=True,
                    stop=True,
                    tile_position=(0, 0),
                )
                nc.tensor.matmul(
                    ps_t[:, SBW : 2 * SBW],
                    kt[64:128, j * P : (j + 1) * P],
                    qt[64:128, :],
                    start=True,
                    stop=True,
                    tile_position=(64, 0),
                )
                ex = epool.tile([P, 2 * SBW], BF16, tag="exp")
                nc.scalar.activation(
                    ex[:], ps_t[:], mybir.ActivationFunctionType.Exp,
                    scale=float(1.0 / np.sqrt(DR)),
                )
                ex_q[q] = ex
                if tt == DEFER_ITERS:
                    epilogue_part2()
            if q >= PREF:
                qa = q - PREF
                sba, ta = qa // NT, qa % NT
                ga, ja = ta // GT, ta % GT
                if ta == 0:
                    po_cur[0] = pop.tile([65, SBW], FP32, tag="po", name="po0")
                    po_cur[1] = pop.tile([65, SBW], FP32, tag="po", name="po1")
                po0, po1 = po_cur
                vs, ex = v_g[ga], ex_q.pop(qa)
                nc.tensor.matmul(
                    po0[:],
                    vs[:, ja * 130 : ja * 130 + 65],
                    ex[:, 0:SBW],
                    start=(ta == 0),
                    stop=(ta == NT - 1),
                )
                nc.tensor.matmul(
                    po1[:],
                    vs[:, ja * 130 + 65 : ja * 130 + 130],
                    ex[:, SBW : 2 * SBW],
                    start=(ta == 0),
                    stop=(ta == NT - 1),
                )
                if ta == NT - 1:
                    epilogue_part1(sba, po0, po1)
        epilogue_part2()

    _split_sync_waits(nc)
    return nc


_program = None


def _get_program():
    global _program
    if _program is None:
        _program = _build_program()
    return _program


def _make_in_maps(x, Wq, Wk, Wv, Wo):
    import ml_dtypes

    bf16 = ml_dtypes.bfloat16
    xts = [np.ascontiguousarray(x[b].T).astype(bf16) for b in range(B)]
    in_maps = []
    for c in range(N_CORES):
        b = c // 4
        hp = c % 4
        h0, h1 = 2 * hp, 2 * hp + 1
        in_maps.append(
            {
                "xt": xts[b],
                "wq": np.ascontiguousarray(
                    np.concatenate([Wq[h0], Wq[h1]], axis=1)
                ).astype(bf16),
                "wk": np.ascontiguousarray(
                    np.concatenate([Wk[h0], Wk[h1]], axis=1)
                ).astype(bf16),
                "wv": np.ascontiguousarray(
                    np.concatenate([Wv[h0], Wv[h1]], axis=1)
                ).astype(bf16),
                "wo": np.ascontiguousarray(Wo[hp * 128 : (hp + 1) * 128]).astype(
                    bf16
                ),
            }
        )
    return in_maps


def kernel(**inputs) -> np.ndarray:
    x = np.asarray(inputs["x"], dtype=np.float32)
    Wq = np.asarray(inputs["Wq"], dtype=np.float32)
    Wk = np.asarray(inputs["Wk"], dtype=np.float32)
    Wv = np.asarray(inputs["Wv"], dtype=np.float32)
    Wo = np.asarray(inputs["Wo"], dtype=np.float32)
    bo = np.asarray(inputs["bo"], dtype=np.float32)

    nc = _get_program()
    in_maps = _make_in_maps(x, Wq, Wk, Wv, Wo)
    res = run_bass_kernel_spmd(nc, in_maps, list(range(N_CORES)))

    y = np.zeros((B, S, D), dtype=np.float32)
    for c in range(N_CORES):
        y[c // 4] += res.results[c]["y"]
    y += bo[None, None, :]
    return y


# revision 24
# speedup vs baseline: 1.0313x; 1.0036x over previous
"""Multi-head attention (B=2, S=4096, D=512, H=8, DR=64) on 8 trn2 NeuronCores.

Sharding: core c -> batch b = c // 4, head-pair hp = c % 4 (heads 2*hp, 2*hp+1).
Each core computes, for its batch and its two heads:
    q/k/v projections, flash-style attention (scores kept on-chip in
    transposed [t, s] orientation so softmax row-sums come from a fused
    ones-column in the AV matmul), and the partial output projection
    y_part = concat(out_h0, out_h1) @ Wo[rows of those heads].
Host sums the 4 partials per batch and adds the bias.

Matmul operands are cast to bf16 on-chip (fp32 matmuls on trn2 run as two
LOW/HIGH passes with an un-hidden LDWEIGHTS between them - ~3x the cost of a
bf16 matmul). All accumulation stays fp32 in PSUM; exp() runs on the fp32
scores; softmax denominators are exact sums of the quantized bf16 weights, so
the attention rows still sum to 1.

The input pipeline (x load/cast/transpose + q/k/v projections + v transpose)
is emitted in 8 groups of 512 s-columns, interleaved with the first
attention block's t-loop, so the ScalarE exp stream starts after one group
instead of after the whole prologue. Per-group SBUF tiles give the Tile
scheduler the dataflow to overlap group g+1's production with attention over
group g.
"""

import sys

for _p in ("/opt/trn_rl_repo", "/root/.axon_site/_ro/trn_rl_repo"):
    if _p not in sys.path:
        sys.path.insert(0, _p)

import numpy as np
from contextlib import ExitStack

import concourse.bass as bass
import concourse.tile as tile
import concourse.mybir as mybir
from concourse.bass_utils import run_bass_kernel_spmd
from concourse.masks import make_identity

B, S, D = 2, 4096, 512
H, DR = 8, 64
P = 128
NT = S // P          # 32 t-tiles (also s-tiles)
SBW = 512            # s-block width
NSB = S // SBW       # 8 s-blocks / t-groups
DC = D // P          # 4 d-chunks
GT = SBW // P        # 4 t-tiles per group
N_CORES = 8
FP32 = mybir.dt.float32
BF16 = mybir.dt.bfloat16

_drain_patched = False


def _patch_tile_drain():
    """This walrus build rejects >1 sync wait on one instruction, which breaks
    TileContext's kernel-tail drain. Spread the waits over nop instructions
    emitted just before the drain."""
    global _drain_patched
    if _drain_patched:
        return
    _drain_patched = True

    def patched(self, tick_clock, wait_clock):
        nop0 = self.nc.sync.nop()
        wait_clock.add_sem_waits(
            nop0.ins, tile.ScopedClock({None: tick_clock.global_clock})
        )
        si = nop0.ins.sync_info
        waits = list(si.on_wait) if si is not None else []
        if waits:
            nop0.ins.sync_info = mybir.SyncInfo(on_wait=waits[:1], on_update=[])
            for w in waits[1:]:
                nop = self.nc.sync.nop()
                nop.ins.sync_info = mybir.SyncInfo(on_wait=[w], on_update=[])
        self.nc.sync.drain()
        self.nc.all_engine_barrier()
        popped = self.nc._tile_sem_poison_stack.pop()
        assert popped is self._sem_poison
        self.nc.clear_and_free_semaphores(list(self.sems.allocated().values()))
        self.nc.all_engine_barrier()

    tile.TileContext._drain_and_barrier = patched


# This walrus build supports only one sync-wait slot per instruction, while
# Tile's sem-assigner attaches up to ~3. Spread the excess onto NoOp
# instructions inserted immediately before the owning instruction (same
# engine, so the stall point is identical and no deadlock can be introduced).
_WAIT_LIMIT = 1
_SKIP_OPCODES = {"AllEngineBarrier", "EventSemaphore", "Call"}


def _split_sync_waits(nc: bass.Bass):
    noop_cls = getattr(mybir, "InstNoOp", None)
    if noop_cls is None:
        import bass_rust

        noop_cls = bass_rust.InstNoOp
    counter = [0]
    for f in nc.m.functions:
        for blk in f.blocks:
            insts = blk.instructions
            new_list = []
            changed = False
            for inst in insts:
                si = inst.sync_info
                waits = list(si.on_wait) if si is not None and si.on_wait else []
                if (
                    len(waits) > _WAIT_LIMIT
                    and inst.opcode not in _SKIP_OPCODES
                    and all(w.sync_type == "semaphore" for w in waits)
                ):
                    excess = waits[: len(waits) - _WAIT_LIMIT]
                    keep = waits[len(waits) - _WAIT_LIMIT :]
                    for w in excess:
                        counter[0] += 1
                        new_list.append(
                            noop_cls(
                                name=f"I-waitsplit-{counter[0]}",
                                engine=inst.engine,
                                debug=inst.debug,
                                ins=[],
                                outs=[],
                                sync_info=mybir.SyncInfo(
                                    on_wait=[w], on_update=[]
                                ),
                            )
                        )
                    inst.sync_info = mybir.SyncInfo(
                        on_wait=keep, on_update=list(si.on_update or [])
                    )
                    changed = True
                new_list.append(inst)
            if changed:
                insts.clear()
                insts.extend(new_list)


def _build_program() -> bass.Bass:
    _patch_tile_drain()
    nc = bass.Bass()

    xt_d = nc.declare_dram_parameter("xt", [D, S], BF16, isOutput=False)
    wq_d = nc.declare_dram_parameter("wq", [D, P], BF16, isOutput=False)
    wk_d = nc.declare_dram_parameter("wk", [D, P], BF16, isOutput=False)
    wv_d = nc.declare_dram_parameter("wv", [D, P], BF16, isOutput=False)
    wo_d = nc.declare_dram_parameter("wo", [P, D], BF16, isOutput=False)
    y_d = nc.declare_dram_parameter("y", [S, D], FP32, isOutput=True)

    with tile.TileContext(nc) as tc, ExitStack() as ctx:
        consts = ctx.enter_context(tc.tile_pool(name="consts", bufs=1))
        wpool = ctx.enter_context(tc.tile_pool(name="weights", bufs=1))
        big = ctx.enter_context(tc.tile_pool(name="big", bufs=1))
        aux = ctx.enter_context(tc.tile_pool(name="aux", bufs=2, space="PSUM"))
        psp = ctx.enter_context(tc.tile_pool(name="ps", bufs=2, space="PSUM"))
        pop = ctx.enter_context(tc.tile_pool(name="po", bufs=2, space="PSUM"))
        epool = ctx.enter_context(tc.tile_pool(name="exp", bufs=11))
        spool = ctx.enter_context(tc.tile_pool(name="small", bufs=4))
        opool = ctx.enter_context(tc.tile_pool(name="osb", bufs=3))
        ypool = ctx.enter_context(tc.tile_pool(name="yout", bufs=3))

        ones64 = consts.tile([1, 64], FP32)
        nc.vector.memset(ones64[:], 1.0)

        # PE warm-up: dense junk matmuls during the initial DMA-bound window
        # keep the HAM clock-gate at 8/8 so the first real matmuls run at
        # 2.4 GHz instead of 1.2 GHz.
        warm = consts.tile([P, D], BF16)
        nc.vector.memset(warm[:], 0.0)
        pw = aux.tile([P, D], FP32, tag="aux", name="pw")
        for _ in range(24):
            nc.tensor.matmul(
                pw[:], warm[:, 0:P], warm[:], start=True, stop=True
            )

        # Weights in bf16; w*_b[p, c*128 + e] = W[c*128 + p, e]
        wq_b = wpool.tile([P, D], BF16)
        wk_b = wpool.tile([P, D], BF16)
        wv_b = wpool.tile([P, D], BF16)
        wo_b = wpool.tile([P, D], BF16)
        for w_b, w_dram in ((wq_b, wq_d), (wk_b, wk_d), (wv_b, wv_d)):
            nc.gpsimd.dma_start(
                w_b[:].rearrange("p (c e) -> p c e", c=DC),
                w_dram[:].rearrange("(c p) e -> p c e", p=P),
            )
        nc.gpsimd.dma_start(wo_b[:], wo_d[:])

        # Per-group persistent tiles (bufs=NSB so every group stays live).
        # xT_g[g][p, c*512 + j] = x[g*512 + j, c*128 + p]
        xtp = ctx.enter_context(tc.tile_pool(name="xtg", bufs=NSB))
        ktp = ctx.enter_context(tc.tile_pool(name="ktg", bufs=NSB))
        qtp = ctx.enter_context(tc.tile_pool(name="qtg", bufs=NSB))
        vtp = ctx.enter_context(tc.tile_pool(name="vtg", bufs=2))
        vsp = ctx.enter_context(tc.tile_pool(name="vsg", bufs=NSB))
        xT_g = [None] * NSB
        kT_g = [None] * NSB   # [e(h0|h1), 512 t-cols]
        qT_g = [None] * NSB   # [e(h0|h1), 512 s-cols]
        v_g = [None] * NSB    # per t-tile in group: [t, 65*2] = [vh0|1 | vh1|1]

        def produce_x_q(g):
            xt = xtp.tile([P, DC * SBW], BF16, tag="xt")
            xT_g[g] = xt
            for c in range(DC):
                eng = nc.sync
                eng.dma_start(
                    xt[:, c * SBW : (c + 1) * SBW],
                    xt_d[c * P : (c + 1) * P, g * SBW : (g + 1) * SBW],
                )
            qt = qtp.tile([P, SBW], BF16, tag="qt")
            qT_g[g] = qt
            pp = aux.tile([P, SBW], FP32, tag="aux")
            for c in range(DC):
                nc.tensor.matmul(
                    pp[:],
                    wq_b[:, c * P : (c + 1) * P],
                    xt[:, c * SBW : (c + 1) * SBW],
                    start=(c == 0),
                    stop=(c == DC - 1),
                )
            nc.vector.tensor_copy(qt[:], pp[:])

        def produce_k(g):
            xt = xT_g[g]
            kt = ktp.tile([P, SBW], BF16, tag="kt")
            kT_g[g] = kt
            pp = aux.tile([P, SBW], FP32, tag="aux")
            for c in range(DC):
                nc.tensor.matmul(
                    pp[:],
                    wk_b[:, c * P : (c + 1) * P],
                    xt[:, c * SBW : (c + 1) * SBW],
                    start=(c == 0),
                    stop=(c == DC - 1),
                )
            nc.vector.tensor_copy(kt[:], pp[:])

        def produce_v_half(g, half):
            xt = xT_g[g]
            if half == 0:
                vs = vsp.tile([P, GT * 130], BF16, tag="vs")
                v_g[g] = vs
            else:
                vs = v_g[g]
            for j in (0, 1) if half == 0 else (2, 3):
                pv = aux.tile([P, P], FP32, tag="aux")
                for c in range(DC):
                    nc.tensor.matmul(
                        pv[:],
                        xt[:, c * SBW + j * P : c * SBW + (j + 1) * P],
                        wv_b[:, c * P : (c + 1) * P],
                        start=(c == 0),
                        stop=(c == DC - 1),
                    )
                dstv = vs[:, j * 130 : j * 130 + 130].rearrange(
                    "p (h q) -> p h q", h=2
                )[:, :, 0:64]
                nc.vector.tensor_copy(
                    dstv, pv[:].rearrange("p (h q) -> p h q", h=2)
                )
            if half == 1:
                ones_cols = vs[:].rearrange("p (t q) -> p t q", t=GT)[
                    :, :, 64:130:65
                ]
                nc.vector.memset(ones_cols, 1.0)

        def produce_group(g):
            produce_x_q(g)
            produce_k(g)
            produce_v_half(g, 0)
            produce_v_half(g, 1)

        # ---- attention + output projection ----
        # Epilogue part 1 (right after a block's t-loop): copy softmax sums
        # and unnormalized bf16 outputs out of PSUM so the po accumulators
        # free immediately. Part 2 (deferred into the next block's t-loop):
        # broadcast sums via PE, one exact reciprocal, normalize, project.
        DEFER_ITERS = 8
        pending = [None]

        def epilogue_part1(sb, po0, po1):
            s0 = spool.tile([1, SBW], FP32, tag="r")
            s1 = spool.tile([1, SBW], FP32, tag="r")
            nc.vector.tensor_copy(s0[:], po0[64:65, :])
            nc.vector.tensor_copy(s1[:], po1[64:65, :])
            osb_u = opool.tile([P, SBW], BF16, tag="osb")
            nc.vector.tensor_copy(osb_u[0:64, :], po0[0:64, :])
            nc.vector.tensor_copy(osb_u[64:128, :], po1[0:64, :])
            pending[0] = (sb, s0, s1, osb_u)

        def epilogue_part2():
            if pending[0] is None:
                return
            sb, s0, s1, osb_u = pending[0]
            pending[0] = None
            pb_t = aux.tile([P, SBW], FP32, tag="aux")
            nc.tensor.matmul(
                pb_t[0:64, :], ones64[:], s0[:],
                start=True, stop=True, tile_position=(0, 0),
            )
            nc.tensor.matmul(
                pb_t[64:128, :], ones64[:], s1[:],
                start=True, stop=True, tile_position=(0, 64),
            )
            bc = spool.tile([P, SBW], FP32, tag="bc")
            nc.vector.tensor_copy(bc[:], pb_t[:])
            osb = opool.tile([P, SBW], BF16, tag="osb")
            for st in range(SBW // P):
                sl = slice(st * P, (st + 1) * P)
                rc = spool.tile([P, P], FP32, tag="rc")
                nc.vector.reciprocal(rc[:], bc[:, sl])
                rcb = spool.tile([P, P], BF16, tag="rcb")
                nc.vector.tensor_copy(rcb[:], rc[:])
                nc.vector.tensor_mul(osb[:, sl], osb_u[:, sl], rcb[:])
                py_t = aux.tile([P, D], FP32, tag="aux")
                nc.tensor.matmul(
                    py_t[:],
                    osb[:, sl],
                    wo_b[:],
                    start=True,
                    stop=True,
                )
                ysb = ypool.tile([P, D], FP32, tag="y")
                nc.vector.tensor_copy(ysb[:], py_t[:])
                row = (sb * (SBW // P) + st) * P
                nc.sync.dma_start(y_d[row : row + P, :], ysb[:])

        produce_group(0)
        produce_group(1)

        PREF = 3
        SPLICE = {}
        for _g in range(2, NSB):
            base = 2 + (_g - 2) * 4
            SPLICE[base] = lambda g=_g: produce_x_q(g)
            SPLICE[base + 1] = lambda g=_g: produce_k(g)
            SPLICE[base + 2] = lambda g=_g: produce_v_half(g, 0)
            SPLICE[base + 3] = lambda g=_g: produce_v_half(g, 1)
        NQ = NSB * NT
        po_cur = [None, None]
        ex_q = {}
        for q in range(NQ + PREF):
            if q < NQ:
                sb, tt = q // NT, q % NT
                g, j = tt // GT, tt % GT
                if sb == 0 and tt in SPLICE:
                    SPLICE[tt]()
                kt, qt = kT_g[g], qT_g[sb]
                ps_t = psp.tile([P, 2 * SBW], FP32, tag="ps")
                nc.tensor.matmul(
                    ps_t[:, 0:SBW],
                    kt[0:64, j * P : (j + 1) * P],
                    qt[0:64, :],
                    start=True,
                    stop=True,
                    tile_position=(0, 0),
                )
                nc.tensor.matmul(
                    ps_t[:, SBW : 2 * SBW],
                    kt[64:128, j * P : (j + 1) * P],
                    qt[64:128, :],
                    start=True,
                    stop=True,
                    tile_position=(64, 0),
                )
                ex = epool.tile([P, 2 * SBW], BF16, tag="exp")
                nc.scalar.activation(
                    ex[:], ps_t[:], mybir.ActivationFunctionType.Exp,
                    scale=float(1.0 / np.sqrt(DR)),
                )
                ex_q[q] = ex
                if tt == DEFER_ITERS:
                    epilogue_part2()
            if q >= PREF:
                qa = q - PREF
                sba, ta = qa // NT, qa % NT
                ga, ja = ta // GT, ta % GT
                if ta == 0:
                    po_cur[0] = pop.tile([65, SBW], FP32, tag="po", name="po0")
                    po_cur[1] = pop.tile([65, SBW], FP32, tag="po", name="po1")
                po0, po1 = po_cur
                vs, ex = v_g[ga], ex_q.pop(qa)
                nc.tensor.matmul(
                    po0[:],
                    vs[:, ja * 130 : ja * 130 + 65],
                    ex[:, 0:SBW],
                    start=(ta == 0),
                    stop=(ta == NT - 1),
                )
                nc.tensor.matmul(
                    po1[:],
                    vs[:, ja * 130 + 65 : ja * 130 + 130],
                    ex[:, SBW : 2 * SBW],
                    start=(ta == 0),
                    stop=(ta == NT - 1),
                )
                if ta == NT - 1:
                    epilogue_part1(sba, po0, po1)
        epilogue_part2()

    _split_sync_waits(nc)
    return nc


_program = None


def _get_program():
    global _program
    if _program is None:
        _program = _build_program()
    return _program


def _make_in_maps(x, Wq, Wk, Wv, Wo):
    import ml_dtypes

    bf16 = ml_dtypes.bfloat16
    xts = [np.ascontiguousarray(x[b].T).astype(bf16) for b in range(B)]
    in_maps = []
    for c in range(N_CORES):
        b = c // 4
        hp = c % 4
        h0, h1 = 2 * hp, 2 * hp + 1
        in_maps.append(
            {
                "xt": xts[b],
                "wq": np.ascontiguousarray(
                    np.concatenate([Wq[h0], Wq[h1]], axis=1)
                ).astype(bf16),
                "wk": np.ascontiguousarray(
                    np.concatenate([Wk[h0], Wk[h1]], axis=1)
                ).astype(bf16),
                "wv": np.ascontiguousarray(
                    np.concatenate([Wv[h0], Wv[h1]], axis=1)
                ).astype(bf16),
                "wo": np.ascontiguousarray(Wo[hp * 128 : (hp + 1) * 128]).astype(
                    bf16
                ),
            }
        )
    return in_maps


def kernel(**inputs) -> np.ndarray:
    x = np.asarray(inputs["x"], dtype=np.float32)
    Wq = np.asarray(inputs["Wq"], dtype=np.float32)
    Wk = np.asarray(inputs["Wk"], dtype=np.float32)
    Wv = np.asarray(inputs["Wv"], dtype=np.float32)
    Wo = np.asarray(inputs["Wo"], dtype=np.float32)
    bo = np.asarray(inputs["bo"], dtype=np.float32)

    nc = _get_program()
    in_maps = _make_in_maps(x, Wq, Wk, Wv, Wo)
    res = run_bass_kernel_spmd(nc, in_maps, list(range(N_CORES)))

    y = np.zeros((B, S, D), dtype=np.float32)
    for c in range(N_CORES):
        y[c // 4] += res.results[c]["y"]
    y += bo[None, None, :]
    return y
